# revision 10
# baseline (speedup 1.0000x reference)
"""DeepSGC (3-layer SGConv, k=2 each) on 8 Trainium2 NeuronCores.

Strategy
--------
The 6 propagation hops h <- D^-1/2 A D^-1/2 h dominate. We fold the
symmetric norm into per-edge weights w_e = norm[dst]*norm[src] (static
graph structure, host-precomputed), making each hop a pure weighted
SpMM. Per core (nodes are sharded 12500/core, padded to 12544 slots):

  for each 128-edge chunk:  PSUM[dst_tile] += S_chunk^T @ table[src_chunk]

where S_chunk is a [128 edges x 128 dst-slots] one-hot-weighted bf16
matrix streamed from HBM, and table[src] rows (bf16, 256B) come from
`dma_gather`. Edges are pre-sorted by (dst tile, src group); 4 src
groups of 25088 rows keep gather indices within int16. Chunk capacities
per (tile, group) are the max over cores so one SPMD program serves all
8 cores. After each hop an AllGather republishes the sharded h as the
full gather table. W matmuls run between hops via PE transpose.
"""
import hashlib
import numpy as np
import ml_dtypes

BF16 = ml_dtypes.bfloat16

# ---------------------------------------------------------------- dims

class Dims:
    def __init__(self, n_nodes, e_edges, nodes_per_core=None, piece_tiles=5):
        self.N = n_nodes
        self.E = e_edges
        self.F = 128
        self.NCLS = 40
        self.NCORES = 8
        self.NGROUPS = 4
        self.TILE = 128
        self.NODES_PER_CORE = nodes_per_core or n_nodes // self.NCORES
        self.TILES_PER_CORE = -(-self.NODES_PER_CORE // self.TILE)
        self.SLOTS_PER_CORE = self.TILES_PER_CORE * self.TILE
        self.TABLE_ROWS = self.SLOTS_PER_CORE * self.NCORES
        self.GROUP_ROWS = self.SLOTS_PER_CORE * (self.NCORES // self.NGROUPS)
        assert self.GROUP_ROWS < 32768
        self.PIECE_TILES = piece_tiles
        self.NPIECES = -(-self.TILES_PER_CORE // piece_tiles)
        self.KLIST = [2, 2, 2]
        self.NHOPS = sum(self.KLIST)


REAL = Dims(100000, 1600000, piece_tiles=5)

# ---------------------------------------------------------------- plan

def build_plan(dm, src, dst):
    src = np.asarray(src, dtype=np.int64)
    dst = np.asarray(dst, dtype=np.int64)
    N, TILE, NG = dm.N, dm.TILE, dm.NGROUPS

    deg = np.bincount(dst, minlength=N).astype(np.float64)
    norm = 1.0 / np.sqrt(np.clip(deg, 1.0, None))
    w_edge = (norm[dst] * norm[src]).astype(np.float32)

    def to_slot(v):
        c = np.minimum(v // dm.NODES_PER_CORE, dm.NCORES - 1)
        return c, c * dm.SLOTS_PER_CORE + (v - c * dm.NODES_PER_CORE)

    c_src, slot_src = to_slot(src)
    c_dst, slot_dst = to_slot(dst)
    g_src = slot_src // dm.GROUP_ROWS
    src_loc = (slot_src - g_src * dm.GROUP_ROWS).astype(np.int32)

    tile_of = ((slot_dst % dm.SLOTS_PER_CORE) // TILE).astype(np.int32)
    slotm = ((slot_dst % dm.SLOTS_PER_CORE) % TILE).astype(np.int32)
    key = (c_dst * dm.TILES_PER_CORE + tile_of) * NG + g_src
    counts = np.bincount(key, minlength=dm.NCORES * dm.TILES_PER_CORE * NG)
    counts = counts.reshape(dm.NCORES, dm.TILES_PER_CORE, NG)
    cap = np.ceil(counts.max(axis=0) / TILE).astype(np.int64)  # [t, g]

    # schedule: chunks ordered (piece, group, tile-in-piece, k)
    pieces = []
    chunk_of_bucket = {}
    ptr = 0
    for p in range(dm.NPIECES):
        t0, t1 = p * dm.PIECE_TILES, min((p + 1) * dm.PIECE_TILES, dm.TILES_PER_CORE)
        base = ptr
        groups = []
        for g in range(NG):
            gstart = ptr
            for t in range(t0, t1):
                chunk_of_bucket[(t, g)] = (ptr, int(cap[t, g]))
                ptr += int(cap[t, g])
            groups.append((gstart, ptr - gstart))
        pieces.append({"t0": t0, "t1": t1, "base": base, "nchunks": ptr - base,
                       "groups": groups})
    C = ptr

    cores = []
    for c in range(dm.NCORES):
        m = c_dst == c
        et, eg = tile_of[m], g_src[m]
        eslotm, esrc, ew = slotm[m], src_loc[m], w_edge[m]
        bkey = et * NG + eg
        idx_sorted = np.argsort(bkey, kind="stable")
        bk = bkey[idx_sorted]
        first = np.r_[True, bk[1:] != bk[:-1]] if len(bk) else np.array([], bool)
        grp_start = np.flatnonzero(first)
        offsets = np.arange(len(bk)) - np.repeat(
            grp_start, np.diff(np.r_[grp_start, len(bk)]))
        rank = np.empty_like(offsets)
        rank[idx_sorted] = offsets
        base = np.array([chunk_of_bucket[(int(t), int(g))][0]
                         for t, g in zip(et, eg)], dtype=np.int64)
        pos = base * TILE + rank

        nslots = C * TILE
        sidx = np.zeros(nslots, dtype=np.int16)
        sslot = np.zeros(nslots, dtype=np.int16)
        sw = np.zeros(nslots, dtype=np.float32)
        sidx[pos] = esrc.astype(np.int16)
        sslot[pos] = eslotm.astype(np.int16)
        sw[pos] = ew
        cores.append({"sidx": sidx, "sslot": sslot, "sw": sw})

    return {"cap": cap, "pieces": pieces, "C": C, "cores": cores}


def build_in_maps(dm, plan, features, W1, b1, W2, b2, W3, b3):
    C = plan["C"]
    in_maps = []
    w1 = np.asarray(W1, np.float32).astype(BF16)
    w2 = np.asarray(W2, np.float32).astype(BF16)
    w3 = np.asarray(W3, np.float32).astype(BF16)
    bias1 = np.ascontiguousarray(
        np.broadcast_to(np.asarray(b1, np.float32), (128, dm.F)))
    bias2 = np.ascontiguousarray(
        np.broadcast_to(np.asarray(b2, np.float32), (128, dm.F)))
    bias3 = np.ascontiguousarray(
        np.broadcast_to(np.asarray(b3, np.float32), (128, dm.NCLS)))
    ident = np.eye(128, dtype=np.float32).astype(BF16)

    feats = np.asarray(features, np.float32)
    table = np.zeros((dm.TABLE_ROWS, dm.F), BF16)
    for c in range(dm.NCORES):
        lo = c * dm.NODES_PER_CORE
        n = min(dm.NODES_PER_CORE, dm.N - lo)
        table[c * dm.SLOTS_PER_CORE: c * dm.SLOTS_PER_CORE + n] = feats[lo:lo + n]

    for c in range(dm.NCORES):
        d = plan["cores"][c]
        # idx wrap: [128, C*8], col ch*8+s, row p -> sidx[ch*128 + s*16 + p%16]
        sidxr = d["sidx"].reshape(C, 8, 16)
        wrap16 = sidxr.transpose(2, 0, 1).reshape(16, C * 8)
        idx_arr = np.tile(wrap16, (8, 1))
        # smat: [128, C*128]
        smat = np.zeros((128, C * 128), BF16)
        pos = np.flatnonzero(d["sw"] != 0)
        if len(pos):
            smat[pos % 128, (pos // 128) * 128 + d["sslot"][pos]] = d["sw"][pos]
        # padded rows of partially-filled chunks also need nothing (zeros)
        in_maps.append({
            "feats": table, "sidx": np.ascontiguousarray(idx_arr), "smat": smat,
            "w1": w1, "w2": w2, "w3": w3,
            "bias1": bias1, "bias2": bias2, "bias3": bias3, "ident": ident,
        })
    return in_maps


# ---------------------------------------------------------------- program

def build_program(dm, plan, linearize=False, nhops=None, fake_gather=False, skip_ag=False):
    import concourse.bass as bass
    import concourse.bacc as bacc
    import concourse.mybir as mybir
    import concourse.tile as tile

    C = plan["C"]
    cap = plan["cap"]
    pieces = plan["pieces"]
    max_pc = max(p["nchunks"] for p in pieces)
    bf16, f32, i16 = mybir.dt.bfloat16, mybir.dt.float32, mybir.dt.int16

    nc = bacc.Bacc("TRN2", target_bir_lowering=False, debug=False,
                   num_devices=dm.NCORES)

    feats_in = nc.dram_tensor("feats", [dm.TABLE_ROWS, dm.F], bf16,
                              kind="ExternalInput")
    sidx_in = nc.dram_tensor("sidx", [128, C * 8], i16, kind="ExternalInput")
    smat_in = nc.dram_tensor("smat", [128, C * 128], bf16, kind="ExternalInput")
    w_in = {k: nc.dram_tensor(k, [128, dm.F], bf16, kind="ExternalInput")
            for k in ("w1", "w2")}
    w_in["w3"] = nc.dram_tensor("w3", [128, dm.NCLS], bf16, kind="ExternalInput")
    b_in = {k: nc.dram_tensor(k, [128, dm.F], f32, kind="ExternalInput")
            for k in ("bias1", "bias2")}
    b_in["bias3"] = nc.dram_tensor("bias3", [128, dm.NCLS], f32,
                                   kind="ExternalInput")
    ident_in = nc.dram_tensor("ident", [128, 128], bf16, kind="ExternalInput")
    out_dram = nc.dram_tensor("out", [dm.SLOTS_PER_CORE, dm.NCLS], f32,
                              kind="ExternalOutput")

    with tile.TileContext(nc, linearize=linearize) as tc:
        with (
            tc.tile_pool(name="const", bufs=1) as constp,
            tc.tile_pool(name="idx", bufs=2) as idxp,
            tc.tile_pool(name="smat", bufs=2) as sp,
            tc.tile_pool(name="gbuf", bufs=2) as gp,
            tc.tile_pool(name="stage", bufs=2) as stp,
            tc.tile_pool(name="small", bufs=3) as smallp,
            tc.tile_pool(name="acc", bufs=4, space="PSUM") as accp,
            tc.tile_pool(name="ptrans", bufs=2, space="PSUM") as ptp,
            tc.tile_pool(name="pz", bufs=2, space="PSUM") as pzp,
            tc.tile_pool(name="dram", bufs=1, space="DRAM") as dramp,
        ):
            w_sb = {}
            for k, t_in in w_in.items():
                w_sb[k] = constp.tile(list(t_in.shape), bf16, tag=k, name=k + "_sb")
                nc.sync.dma_start(w_sb[k][:], t_in[:])
            b_sb = {}
            for k, t_in in b_in.items():
                b_sb[k] = constp.tile(list(t_in.shape), f32, tag=k, name=k + "_sb")
                nc.sync.dma_start(b_sb[k][:], t_in[:])
            ident_sb = constp.tile([128, 128], bf16, tag="ident")
            nc.sync.dma_start(ident_sb[:], ident_in[:])

            shard = dramp.tile([dm.SLOTS_PER_CORE, dm.F], bf16, tag="shard")
            tables = [
                dramp.tile([dm.TABLE_ROWS, dm.F], bf16, tag=f"table{h}",
                           addr_space="Shared", name=f"table{h}")
                for h in range(1, dm.NHOPS)
            ]

            NH = nhops or dm.NHOPS
            for hop in range(NH):
                tbl = feats_in if hop == 0 else tables[hop - 1]
                layer_end = hop in (1, 3)
                final = hop == NH - 1
                wk = "w3" if final else ("w1", "w2", "w3")[hop // 2]
                bk = "bias3" if final else ("bias1", "bias2", "bias3")[hop // 2]

                for pi, pinfo in enumerate(pieces):
                    t0, t1, base, npc = (pinfo["t0"], pinfo["t1"],
                                         pinfo["base"], pinfo["nchunks"])
                    ntiles = t1 - t0
                    idx_t = idxp.tile([128, max_pc * 8], i16, tag="idx")
                    s_t = sp.tile([128, max_pc * 128], bf16, tag="smat")
                    g_t = gp.tile([128, max_pc, dm.F], bf16, tag="gbuf")
                    nc.sync.dma_start(idx_t[:, :npc * 8],
                                      sidx_in[:, base * 8:(base + npc) * 8])
                    nc.sync.dma_start(s_t[:, :npc * 128],
                                      smat_in[:, base * 128:(base + npc) * 128])
                    for g, (gstart, gn) in enumerate(pinfo["groups"]):
                        if gn == 0:
                            continue
                        lo = gstart - base
                        GMAX = 6  # chunks per dma_gather (SWDGE ring cap)
                        for c0 in range(0, gn, GMAX):
                            cn = min(GMAX, gn - c0)
                            l0 = lo + c0
                            if fake_gather:
                                nc.sync.dma_start(
                                    g_t[:, l0:l0 + cn, :],
                                    tbl[g * dm.GROUP_ROWS:g * dm.GROUP_ROWS + cn * 128, :]
                                    .rearrange("(c p) f -> p c f", p=128))
                            else:
                                nc.gpsimd.dma_gather(
                                    g_t[:, l0:l0 + cn, :],
                                    tbl[g * dm.GROUP_ROWS:(g + 1) * dm.GROUP_ROWS, :],
                                    idx_t[:, l0 * 8:(l0 + cn) * 8],
                                    num_idxs=cn * 128, num_idxs_reg=cn * 128,
                                    elem_size=dm.F,
                                )

                    if final:
                        ost = stp.tile([128, dm.PIECE_TILES * dm.NCLS], f32,
                                       tag="ostf")
                    else:
                        ost = stp.tile([128, dm.PIECE_TILES * dm.F], bf16,
                                       tag="ost")

                    for t in range(t0, t1):
                        ti = t - t0
                        nch = int(cap[t].sum())
                        acc = accp.tile([128, dm.F], f32, tag="acc")
                        first = True
                        done = 0
                        for g in range(dm.NGROUPS):
                            st, capg = None, int(cap[t, g])
                            if capg == 0:
                                continue
                            st = plan_chunk_start(pieces, pi, g, t, cap)
                            for k in range(capg):
                                lo = st - base + k
                                done += 1
                                nc.tensor.matmul(
                                    acc[:], s_t[:, lo * 128:(lo + 1) * 128],
                                    g_t[:, lo, :],
                                    start=first, stop=(done == nch),
                                )
                                first = False

                        if not layer_end and not final:
                            o = ost[:, ti * dm.F:(ti + 1) * dm.F]
                            if nch == 0:
                                nc.vector.memset(o, 0.0)
                            else:
                                nc.vector.tensor_copy(o, acc[:])
                        else:
                            h_sb = smallp.tile([128, dm.F], bf16, tag="hsb")
                            if nch == 0:
                                nc.vector.memset(h_sb[:], 0.0)
                            else:
                                nc.vector.tensor_copy(h_sb[:], acc[:])
                            tp = ptp.tile([128, 128], bf16, tag="tp")
                            nc.tensor.transpose(tp[:], h_sb[:], ident_sb[:])
                            hT = smallp.tile([128, 128], bf16, tag="hT")
                            nc.vector.tensor_copy(hT[:], tp[:])
                            ncols = dm.NCLS if final else dm.F
                            z = pzp.tile([128, ncols], f32, tag="pz")
                            nc.tensor.matmul(z[:], hT[:], w_sb[wk][:],
                                             start=True, stop=True)
                            o = ost[:, ti * ncols:(ti + 1) * ncols]
                            if final:
                                nc.vector.tensor_add(o, z[:], b_sb[bk][:])
                            else:
                                zb = smallp.tile([128, dm.F], f32, tag="zb")
                                nc.vector.tensor_add(zb[:], z[:], b_sb[bk][:])
                                nc.vector.tensor_scalar_max(o, zb[:], 0.0)

                    # piece writeback
                    if final:
                        dview = out_dram[t0 * 128:t1 * 128, :].rearrange(
                            "(t p) f -> p t f", p=128)
                        nc.sync.dma_start(
                            dview, ost[:, :ntiles * dm.NCLS].rearrange(
                                "p (t f) -> p t f", f=dm.NCLS))
                    else:
                        dview = shard[t0 * 128:t1 * 128, :].rearrange(
                            "(t p) f -> p t f", p=128)
                        nc.sync.dma_start(
                            dview, ost[:, :ntiles * dm.F].rearrange(
                                "p (t f) -> p t f", f=dm.F))

                if not final and not skip_ag:
                    nc.gpsimd.collective_compute(
                        "AllGather", mybir.AluOpType.bypass,
                        replica_groups=[list(range(dm.NCORES))],
                        ins=[shard.opt()], outs=[tables[hop].opt()],
                    )

    nc.compile()
    return nc


def plan_chunk_start(pieces, pi, g, t, cap):
    """global chunk index where bucket (t, g) starts."""
    pinfo = pieces[pi]
    gstart = pinfo["groups"][g][0]
    off = 0
    for tt in range(pinfo["t0"], t):
        off += int(cap[tt, g])
    return gstart + off


# ---------------------------------------------------------------- entry

_CACHE = {}


def _run(dm, inputs):
    from concourse.bass_utils import run_bass_kernel_spmd

    src, dst = np.asarray(inputs["src"]), np.asarray(inputs["dst"])
    key = hashlib.md5(src.tobytes() + dst.tobytes()).hexdigest()
    if key not in _CACHE:
        plan = build_plan(dm, src, dst)
        prog = build_program(dm, plan)
        _CACHE[key] = (plan, prog)
    plan, prog = _CACHE[key]

    in_maps = build_in_maps(dm, plan, inputs["features"],
                            inputs["W1"], inputs["b1"], inputs["W2"],
                            inputs["b2"], inputs["W3"], inputs["b3"])
    res = run_bass_kernel_spmd(prog, in_maps, core_ids=list(range(dm.NCORES)))
    out = np.zeros((dm.N, dm.NCLS), np.float32)
    for c in range(dm.NCORES):
        lo = c * dm.NODES_PER_CORE
        n = min(dm.NODES_PER_CORE, dm.N - lo)
        out[lo:lo + n] = np.asarray(res.results[c]["out"])[:n]
    return out


def kernel(features, src, dst, W1, b1, W2, b2, W3, b3):
    return _run(REAL, {"features": features, "src": src, "dst": dst,
                       "W1": W1, "b1": b1, "W2": W2, "b2": b2,
                       "W3": W3, "b3": b3})
